# revision 42
# baseline (speedup 1.0000x reference)
"""Conv1d (K=5, pad=2) with folded LoRA on 8 Trainium2 NeuronCores.

Strategy
--------
Data-parallel: batch 8 -> 1 batch item per core. LoRA is folded into the
conv weights on the host:
    W_eff = conv_w + (alpha/rank) * einsum('or,rik->oik', lora_B, lora_A)
so the device kernel is a single conv1d + bias.

The conv runs in fp8 (e4m3) DoubleRow matmuls: each matmul carries TWO
(weight, x) slot-pairs per partition, contracting 256 elements at 0.5
cycles/row — 4x the fp32r MAC rate. Accuracy is recovered with a 2-slot
pair-quantization: slot A holds the operand at full scale, slot B holds a
1/8-scale corrector chosen jointly so that A + B/8 lands on an ~8x finer
effective grid than e4m3 (host-side search, both operands). Measured
rel-absmax error vs the fp32 reference: ~1.0e-2 (gate: 2e-2).

Scales: x*32 (slot A) / x*4 (slot B), W*256 (both slots).
PSUM = 256*32*(1+1/8) * conv -> eviction does psum*(1/9216) + bias in one
DVE tensor_scalar (mult, add).

Per core: y[co, t] = bias[co] + sum_{k,ci} W_eff[co, ci, k] * x[ci, t+k-2]
computed as 10 DoubleRow matmuls (2 ci-blocks x 5 taps) accumulating in
PSUM per (co-block, 512-column tile).

Toolchain constraint baked into the structure: every instruction may carry
at most ONE sync wait (walrus setupSyncWait limit), and Tile's wait elision
is per-proc. Hence (all inherited from the fp32r version of this kernel):
  - PE "observer" matmuls (2-column, scratch PSUM) absorb each x/weight DMA
    lane wait so real matmuls only wait on the DVE sem (PSUM-bank WAR).
  - Evictions (PSUM->SBUF, descale+bias) run exclusively on DVE and wait
    only on PE; out-DMA-slot WAR is absorbed by tiny DVE memsets; the bias
    lane by a tiny DVE copy.
  - x-loads ride the SP HWDGE ring, stores the ACT/SWDGE rings; each store
    owns its own output tensor + DMASW lane.
  - A tail chain of 1-dep sync nops covers all procs so the exit drain
    carries at most one wait.
"""
import sys
sys.path.insert(0, "/opt/trn_rl_repo")
import numpy as np
import ml_dtypes

from concourse import bass, mybir, tile
from concourse import bass_utils
from concourse.tile import add_dep_helper

E4M3 = ml_dtypes.float8_e4m3  # TRN fp8e4 (max normal 240)

# Problem constants (hardcoded per contract)
B = 8
CI = 256
CO = 256
K = 5
PAD = 2
T = 16384
RANK = 8
ALPHA = 16.0
SCALING = ALPHA / RANK
N_CORES = 8

# Quantization scheme
XS = 32.0        # x slot-A scale
WS = 256.0       # W scale (both slots)
DFRAC = 0.125    # slot-B attenuation: XB ~ x*XS/8, effective W = WA + WB/8
DESCALE = 1.0 / ((1.0 + DFRAC) * XS * WS)

# Tiling
CHUNK = 1024          # output columns per chunk
NCHUNK = T // CHUNK   # 16
SUB = 512             # matmul free dim (one PSUM bank)
NSUB = CHUNK // SUB   # 2
XCOLS = CHUNK + 2 * PAD  # chunk + halo
TP = T + 2 * PAD      # padded x length in DRAM (zero halos baked in)
NBLK = 10             # contraction blocks: 2 ci-blocks x 5 taps


NWARM_DEFAULT = 160


def _build_nc(reps=1, nwarm=None):
    f32 = mybir.dt.float32
    f16 = mybir.dt.float16
    f8 = mybir.dt.float8e4
    DR = mybir.MatmulPerfMode.DoubleRow

    nc = bass.Bass(trn_type="TRN2", debug=False)
    # xq[p, b, slot, t] : X{A,B}[b*128+p, t-2], zero-padded halos
    xq = nc.dram_tensor("xq", [128, 2, 2, TP], f8, kind="ExternalInput").ap()
    # wts[ci, ((co*10 + k*2+b)*2+slot)*128 + m]  (co-major: lets the first
    # co-block's weights arrive before the full tensor)
    wts = nc.dram_tensor("wts", [128, NBLK * 2 * 2 * 128], f8,
                         kind="ExternalInput").ap()
    bias = nc.dram_tensor("bias", [128, 2], f32, kind="ExternalInput").ap()
    # one output tensor per 2048-wide super-chunk, stored via SWDGE so each
    # store owns a DMASW lane exactly once; host concatenates
    # fp16 output halves store traffic; host upcasts (adds ~3e-4 rel err).
    # The last super-chunk is split into 3 tensors so each tail store owns
    # its own DMA lane (single sync wait each).
    ys = [nc.dram_tensor(f"y{s}", [CI, 2 * CHUNK], f16, kind="ExternalOutput").ap()
          for s in range(NCHUNK // 2 - 1)]
    ys.append(nc.dram_tensor("y7a", [CI, CHUNK], f16, kind="ExternalOutput").ap())
    ys.append(nc.dram_tensor("y7b", [CI, SUB], f16, kind="ExternalOutput").ap())
    ys.append(nc.dram_tensor("y7c", [CI, SUB], f16, kind="ExternalOutput").ap())
    ysab = [yc.rearrange("(b p) t -> p b t", p=128) for yc in ys]

    NOB = 8   # out staging buffers: one per super-chunk, never recycled
    NPB = 6   # psum accumulation banks
    WHALF = NBLK * 2 * 128  # weight columns per co-block
    NWARM = NWARM_DEFAULT if nwarm is None else nwarm
    C0SPLIT = 520           # first piece of chunk-0's load (covers ts=0)

    with tile.TileContext(nc) as tc:
        with tc.tile_pool(name="wp", bufs=1) as wp, \
             tc.tile_pool(name="pp", bufs=1, space="PSUM") as pp:

            # write-once observer scratch: two columns per observer matmul;
            # warm_ps shares the same PSUM bank
            obs_ps = pp.tile([128, 64], f32, name="obs_ps", tag="obs")
            warm_ps = pp.tile([128, 64], f32, name="warm_ps", tag="warm")
            pbufs = [pp.tile([128, SUB], f32, name=f"pt{j}", tag=f"pt{j}")
                     for j in range(NPB)]
            # x is fully resident: one dedicated buffer per chunk, no reuse
            xbufs = [wp.tile([128, 2, 2, XCOLS], f8, name=f"xt{j}", tag=f"xt{j}")
                     for j in range(NCHUNK)]
            obufs = [wp.tile([128, 2, 2 * CHUNK], f16, name=f"ot{j}", tag=f"ot{j}")
                     for j in range(NOB)]
            # write-once DVE gate scratch: one column per gate memset
            gs = wp.tile([128, 4 * NCHUNK * reps + 8], f32, name="gs")
            # pool-ring gate scratch (absorbs SWDGE lane-reuse waits)
            gsp = wp.tile([128, 4], f32, name="gsp")

            wr = wp.tile([128, NBLK * 2 * 2 * 128], f8, name="wr")
            bs = wp.tile([128, 2], f32, name="bs")
            # warmup scratch: zeroed by DVE, then NWARM tiny DoubleRow
            # matmuls keep PE continuously busy through the p-state ramp
            # while the first loads land, so real matmuls start at 2.4 GHz
            dum = wp.tile([128, 2, 192], f8, name="dum")
            ms_dum = nc.vector.memset(dum[:], 0.0)
            warm = None
            for _ in range(NWARM):
                warm = nc.tensor.matmul(
                    warm_ps[:], dum[:, :, 0:128], dum[:, :, 128:192],
                    start=True, stop=True,
                    perf_mode=mybir.MatmulPerfMode.DoubleRow)


            n_obs = [0]

            def pe_observe(src_ap, dma_inst):
                """2-column matmul whose only wait is `dma_inst`'s lane.

                Reads only within the region `dma_inst` wrote; writes its own
                never-reused obs_ps column (no WAW chain)."""
                n = src_ap.shape[-1]
                m = min(128, n)
                oc = 2 * n_obs[0]
                n_obs[0] += 1
                mm = nc.tensor.matmul(obs_ps[0:m, oc:oc + 2], src_ap[:, 0:m],
                                      src_ap[:, 0:2], start=True, stop=True)
                add_dep_helper(mm.ins, dma_inst.ins, sync=False, reason="obs-order")
                return mm

            n_gate = [0]

            def dve_gate(dep_inst):
                """Write-once DVE memset whose only wait is dep's proc tick."""
                gc = n_gate[0]
                n_gate[0] += 1
                ms = nc.vector.memset(gs[:, gc:gc + 1], 0.0)
                add_dep_helper(ms.ins, dep_inst.ins, sync=True, reason="dve-gate")
                return ms

            bscratch = wp.tile([128, 2], f32, name="bscratch")
            obs_b = None  # emitted after d_b below

            in_dmas = []      # x-chunk loads
            hw_ring = []      # every HWDGE DMA in issue order (lane = idx%8)
            out_dmas = []     # per super-chunk (final rep only)
            gates = []        # ring-gate instructions needing tail coverage
            sc_evicts = {}    # global super-chunk -> last evict
            sc_ods = {}       # global super-chunk -> out dma
            last_mm = None
            last_evict = None
            pi = 0            # psum bank rotation
            NSC = NCHUNK // 2

            obs_w1 = None
            for r in range(reps):
                for c in range(NCHUNK):
                    xt = xbufs[c]

                    observers = []
                    d_c0b = None
                    if r == 0:
                        if c == 0:
                            # ring order: w(co0), c0-first-piece, c0-rest,
                            # w(co1), bias — each lands just before PE (at
                            # full post-warmup speed) reaches the groups
                            # that need it
                            d_w0 = nc.sync.dma_start(out=wr[:, 0:WHALF],
                                                     in_=wts[:, 0:WHALF])
                            hw_ring.append(d_w0)
                            observers.append(
                                pe_observe(wr[:, 0:WHALF], d_w0))
                            da = nc.sync.dma_start(
                                out=xt[:, :, :, 0:C0SPLIT],
                                in_=xq[:, :, :, 0:C0SPLIT])
                            hw_ring.append(da)
                            in_dmas.append(da)
                            observers.append(
                                pe_observe(xt[:, 0, 0, 0:C0SPLIT], da))
                            d_c0b = nc.sync.dma_start(
                                out=xt[:, :, :, C0SPLIT:XCOLS],
                                in_=xq[:, :, :, C0SPLIT:XCOLS])
                            hw_ring.append(d_c0b)
                            in_dmas.append(d_c0b)
                            d_w1 = nc.sync.dma_start(out=wr[:, WHALF:],
                                                     in_=wts[:, WHALF:])
                            hw_ring.append(d_w1)
                            d_b = nc.sync.dma_start(out=bs[:], in_=bias[:])
                            hw_ring.append(d_b)
                            obs_b = nc.vector.tensor_copy(bscratch[:], bs[:])
                        else:
                            # halo baked into DRAM: one uniform DMA per chunk
                            d = nc.sync.dma_start(
                                out=xt[:],
                                in_=xq[:, :, :, c * CHUNK:c * CHUNK + XCOLS])
                            hw_ring.append(d)
                            in_dmas.append(d)
                            observers.append(pe_observe(xt[:, 0, 0, :], d))

                    sc, half = divmod(c, 2)
                    gsc = r * NSC + sc
                    ot = obufs[gsc % NOB]
                    evict_gates = [obs_b]
                    if half == 0 and gsc >= NOB:
                        # pre-lift the recycled out buffer's history onto
                        # DVE's observed clock: one 1-wait gate per proc
                        old = gsc - NOB
                        evict_gates.append(dve_gate(sc_evicts[old]))
                        if old in sc_ods:
                            evict_gates.append(dve_gate(sc_ods[old]))

                    first_evict_of_chunk = True
                    last_chunk = (r == reps - 1 and c == NCHUNK - 1)
                    if last_chunk:
                        # ts-outer so each 512-col piece finishes across both
                        # co blocks early -> finer-grained final stores
                        group_order = [(0, 0), (1, 0), (0, 1), (1, 1)]
                    else:
                        group_order = [(co, ts) for co in range(2)
                                       for ts in range(NSUB)]
                    for gi, (co, ts) in enumerate(group_order):
                        if r == 0 and c == 0:
                            if co == 1 and obs_w1 is None:
                                # co-1 weights observed between chunk-0's co
                                # halves; d_w1 lands well before PE gets here
                                obs_w1 = pe_observe(wr[:, WHALF:], d_w1)
                                observers = observers + [obs_w1]
                            if ts == 1 and d_c0b is not None:
                                observers = observers + [pe_observe(
                                    xt[:, 0, 0, C0SPLIT:XCOLS], d_c0b)]
                                d_c0b = None
                        if True:
                            pt = pbufs[pi % NPB]
                            pi += 1
                            first = True
                            for b in range(2):
                                for k in range(K):
                                    widx = ((co * NBLK + k * 2 + b) * 2) * 128
                                    mm = nc.tensor.matmul(
                                        pt[:],
                                        wr[:, widx:widx + 256].rearrange(
                                            "p (two m) -> p two m", two=2),
                                        xt[:, b, :,
                                           ts * SUB + k: ts * SUB + k + SUB],
                                        start=first,
                                        stop=(b == 1 and k == K - 1),
                                        perf_mode=DR,
                                    )
                                    if first:
                                        for ob in observers:
                                            add_dep_helper(
                                                mm.ins, ob.ins, sync=False,
                                                reason="order-after-observe")
                                    first = False
                                    last_mm = mm
                            off = half * CHUNK + ts * SUB
                            ev = nc.vector.tensor_scalar(
                                out=ot[:, co, off:off + SUB],
                                in0=pt[:],
                                scalar1=DESCALE,
                                scalar2=bs[:, co:co + 1],
                                op0=mybir.AluOpType.mult,
                                op1=mybir.AluOpType.add,
                            )
                            if first_evict_of_chunk:
                                for g in evict_gates:
                                    add_dep_helper(ev.ins, g.ins, sync=False,
                                                   reason="order-after-gate")
                                first_evict_of_chunk = False
                            last_evict = ev

                    final_sc = (r == reps - 1 and sc == NSC - 1)
                    if final_sc and half == 0:
                        # last super-chunk: store the first chunk's columns as
                        # soon as they're evicted so only 512-col pieces tail
                        od = nc.gpsimd.dma_start(out=ysab[NSC - 1][:],
                                                 in_=ot[:, :, 0:CHUNK])
                        out_dmas.append(od)
                    if half == 1:
                        sc_evicts[gsc] = last_evict
                        if r == reps - 1 and not final_sc:
                            # SWDGE store: own output tensor + own DMASW lane
                            od = nc.gpsimd.dma_start(out=ysab[sc][:], in_=ot[:])
                            sc_ods[gsc] = od
                            out_dmas.append(od)
                        elif final_sc:
                            # 512-col tail pieces. These are SWDGE stores 9
                            # and 10 — their lanes wrap onto stores 0 and 1
                            # (8 DMASW lanes), so a pool-ring gate absorbs
                            # each lane-reuse wait (walrus 1-wait limit).
                            pg1 = nc.gpsimd.memset(gsp[:, 0:1], 0.0)
                            add_dep_helper(pg1.ins, out_dmas[0].ins,
                                           sync=True, reason="lane-gate")
                            gates.append(pg1)
                            od1 = nc.gpsimd.dma_start(
                                out=ysab[NSC][:],
                                in_=ot[:, :, CHUNK:CHUNK + SUB])
                            out_dmas.append(od1)
                            # the very last piece rides the idle ACT HWDGE
                            # ring (lower launch overhead than SWDGE); its
                            # lane wraps onto HWDGE DMA #-8, absorbed by an
                            # ACT-ring gate
                            if len(hw_ring) >= 8:
                                pg2 = nc.scalar.memzero(gsp[:, 1:2])
                                add_dep_helper(pg2.ins,
                                               hw_ring[len(hw_ring) - 8].ins,
                                               sync=True, reason="lane-gate")
                                gates.append(pg2)
                            od2 = nc.scalar.dma_start(
                                out=ysab[NSC + 1][:],
                                in_=ot[:, :, CHUNK + SUB:2 * CHUNK])
                            hw_ring.append(od2)
                            out_dmas.append(od2)
                            sc_ods[gsc] = od2

            # Tail flush: cover every proc with 1-dep sync nops so the final
            # drain carries at most one wait. Ordered by expected completion
            # (the nops run in-order on the sync proc).
            tail_deps = (in_dmas[-8:] + out_dmas[:-2] + gates
                         + [last_mm, last_evict] + out_dmas[-2:])
            for dep in tail_deps:
                nop = nc.sync.nop()
                add_dep_helper(nop.ins, dep.ins, sync=True, reason="tailflush")

    return nc


def check_waits(nc):
    """Return instructions carrying more than one sync wait (walrus limit)."""
    bad = []
    for f in nc.m.functions:
        for bb in f.blocks:
            for inst in bb.instructions:
                si = inst.sync_info
                nw = len(si.on_wait) if si and si.on_wait else 0
                if nw > 1:
                    bad.append((inst.name, type(inst).__name__, nw,
                                [w.ant_name for w in si.on_wait]))
    return bad


def _nudge(q, steps):
    """e4m3 array q moved by `steps` representable values (value order)."""
    bits = q.view(np.uint8)
    mag = (bits & 0x7F).astype(np.int32)
    sign = (bits & 0x80) != 0
    m2 = np.where(sign, mag - steps, mag + steps)
    flip = m2 < 0
    m2 = np.where(flip, -m2, m2)
    s2 = np.where(flip, ~sign, sign)
    m2 = np.clip(m2, 0, 0x7E)
    return ((np.where(s2, 0x80, 0) | m2).astype(np.uint8)).view(E4M3)


def _pair_min(v, b_scale, b_weight, na, nb):
    """Choose e4m3 (A, B) minimizing |A + b_weight*B - (1+b_weight*b_scale)*v|.

    A ~ v, B ~ v*b_scale. Returns (A, B) as e4m3 arrays."""
    v = np.asarray(v, np.float32)
    tgt = (1.0 + b_weight * b_scale) * v
    qa = v.astype(E4M3)
    qb = (v * b_scale).astype(E4M3)
    best = None
    bestA = bestB = None
    for ia in range(-na, na + 1):
        A = _nudge(qa, ia)
        Af = A.astype(np.float32)
        for ib in range(-nb, nb + 1):
            Bv = _nudge(qb, ib)
            err = np.abs(Af + np.float32(b_weight) * Bv.astype(np.float32) - tgt)
            if best is None:
                best, bestA, bestB = err, A.copy(), Bv.copy()
            else:
                m = err < best
                np.copyto(best, err, where=m)
                np.copyto(bestA, A, where=m)
                np.copyto(bestB, Bv, where=m)
    return bestA, bestB


def _pack_weights(conv_w, conv_b, lora_A, lora_B):
    w_eff = conv_w.astype(np.float64) + SCALING * np.einsum(
        "or,rik->oik", lora_B.astype(np.float64),
        lora_A.astype(np.float64).reshape(RANK, CI, K))
    w_eff = w_eff.astype(np.float32)
    # both W slots at full scale; slot B enters the sum with weight 1/8
    # (via x slot B's 1/8 scale), so A + B/8 must track (1+1/8) * w * WS
    wa, wb = _pair_min(w_eff * WS, b_scale=1.0, b_weight=DFRAC, na=1, nb=3)
    # wts[ci, ((co*10 + k*2+b)*2+slot)*128 + m] = W{slot}[co*128+m, b*128+ci, k]
    a = np.stack([np.asarray(wa), np.asarray(wb)])     # [slot, CO, CI, K]
    a = a.reshape(2, 2, 128, 2, 128, K)                # [slot, cob, m, cib, ci, k]
    a = a.transpose(4, 1, 5, 3, 0, 2)                  # [ci, cob, k, cib, slot, m]
    wts = np.ascontiguousarray(a.reshape(128, NBLK * 2 * 2 * 128))
    bias = np.ascontiguousarray(
        conv_b.astype(np.float32).reshape(2, 128).T)   # [128, 2]
    return wts, bias


def _pack_x(x):
    """x [CI, T] fp32 -> xq [128, 2, 2, TP] e4m3 with zero halos."""
    xa, xb = _pair_min(x * XS, b_scale=DFRAC, b_weight=1.0, na=1, nb=2)
    xq = np.zeros((128, 2, 2, TP), dtype=E4M3)
    xq[:, :, 0, PAD:PAD + T] = np.asarray(xa).reshape(2, 128, T).transpose(1, 0, 2)
    xq[:, :, 1, PAD:PAD + T] = np.asarray(xb).reshape(2, 128, T).transpose(1, 0, 2)
    return xq


_CACHED_NC = None


def kernel(x, conv_w, conv_b, lora_A, lora_B, _trace=False):
    global _CACHED_NC
    x = np.asarray(x, dtype=np.float32)
    wts, bias = _pack_weights(np.asarray(conv_w), np.asarray(conv_b),
                              np.asarray(lora_A), np.asarray(lora_B))

    if _CACHED_NC is None:
        _CACHED_NC = _build_nc()
        bad = check_waits(_CACHED_NC)
        assert not bad, f"sync-wait violations: {bad[:5]}"
    nc = _CACHED_NC

    in_maps = [
        {"xq": _pack_x(x[i]), "wts": wts, "bias": bias}
        for i in range(N_CORES)
    ]
    res = bass_utils.run_bass_kernel_spmd(
        nc, in_maps, core_ids=list(range(N_CORES)), trace=_trace)
    names = [f"y{s}" for s in range(NCHUNK // 2 - 1)] + ["y7a", "y7b", "y7c"]
    out = np.stack(
        [np.concatenate(
            [res.results[i][nm].astype(np.float32) for nm in names], axis=1)
         for i in range(N_CORES)], axis=0)
    if _trace:
        kernel._last_exec_time_ns = res.exec_time_ns
        kernel._last_results = res
    return out


if __name__ == "__main__":
    nc = _build_nc()
    bad = check_waits(nc)
    print("violations:", bad[:10])
    n_inst = sum(len(bb.instructions) for f in nc.m.functions for bb in f.blocks)
    print("instructions:", n_inst)


# revision 51
# speedup vs baseline: 1.0049x; 1.0049x over previous
"""Conv1d (K=5, pad=2) with folded LoRA on 8 Trainium2 NeuronCores.

Strategy
--------
Data-parallel: batch 8 -> 1 batch item per core. LoRA is folded into the
conv weights on the host:
    W_eff = conv_w + (alpha/rank) * einsum('or,rik->oik', lora_B, lora_A)
so the device kernel is a single conv1d + bias.

The conv runs in fp8 (e4m3) DoubleRow matmuls: each matmul carries TWO
(weight, x) slot-pairs per partition, contracting 256 elements at 0.5
cycles/row — 4x the fp32r MAC rate. Accuracy is recovered with a 2-slot
pair-quantization: slot A holds the operand at full scale, slot B holds a
1/8-scale corrector chosen jointly so that A + B/8 lands on an ~8x finer
effective grid than e4m3 (host-side search, both operands). Measured
rel-absmax error vs the fp32 reference: ~1.0e-2 (gate: 2e-2).

Scales: x*32 (slot A) / x*4 (slot B), W*256 (both slots).
PSUM = 256*32*(1+1/8) * conv -> eviction does psum*(1/9216) + bias in one
DVE tensor_scalar (mult, add).

Per core: y[co, t] = bias[co] + sum_{k,ci} W_eff[co, ci, k] * x[ci, t+k-2]
computed as 10 DoubleRow matmuls (2 ci-blocks x 5 taps) accumulating in
PSUM per (co-block, 512-column tile).

Schedule (TimelineSim ~78.0us vs 177.5us for the fp32r version; PE floor
for this shape is 68.3us):
  - ~60 warmup matmuls on a zeroed scratch tile keep PE busy through the
    0.65->2.4 GHz p-state ramp while the first loads land.
  - Startup DMA order w(co0) -> chunk0[0:516] -> chunk0[516:] -> w(co1) ->
    bias: each piece lands just before the first groups need it (globally
    serialized DMA bus; descriptor runs are kept >=512B to avoid the 2x
    small-descriptor latency).
  - 8 output staging buffers (one per 2048-col super-chunk, never
    recycled) so no eviction ever waits on a store.
  - fp16 stores halve out-traffic; the last super-chunk is stored as
    1024/512/512-col pieces (the final one on the idle ACT HWDGE ring) to
    shorten the post-compute tail.

Toolchain constraint baked into the structure: every instruction may carry
at most ONE sync wait (walrus setupSyncWait limit), and Tile's wait elision
is per-proc. Hence (inherited from the fp32r version of this kernel):
  - PE "observer" matmuls (2-column, scratch PSUM) absorb each x/weight DMA
    lane wait so real matmuls only wait on the DVE sem (PSUM-bank WAR).
  - Evictions (PSUM->SBUF, descale+bias) run exclusively on DVE and wait
    only on PE; the bias lane is absorbed by a tiny DVE copy.
  - x-loads ride the SP HWDGE ring; stores are SWDGE with one output
    tensor per store. DMA lanes are 8-way round-robin per ring, so the
    9th+ store's lane-reuse wait is absorbed by a same-ring gate memset.
  - A tail chain of 1-dep sync nops covers all procs so the exit drain
    carries at most one wait.
"""
import sys
sys.path.insert(0, "/opt/trn_rl_repo")
import numpy as np
import ml_dtypes

from concourse import bass, mybir, tile
from concourse import bass_utils
from concourse.tile import add_dep_helper

E4M3 = ml_dtypes.float8_e4m3  # TRN fp8e4 (max normal 240)

# Problem constants (hardcoded per contract)
B = 8
CI = 256
CO = 256
K = 5
PAD = 2
T = 16384
RANK = 8
ALPHA = 16.0
SCALING = ALPHA / RANK
N_CORES = 8

# Quantization scheme
XS = 32.0        # x slot-A scale
WS = 256.0       # W scale (both slots)
DFRAC = 0.125    # slot-B attenuation: XB ~ x*XS/8, effective W = WA + WB/8
DESCALE = 1.0 / ((1.0 + DFRAC) * XS * WS)

# Tiling
CHUNK = 1024          # output columns per chunk
NCHUNK = T // CHUNK   # 16
SUB = 512             # matmul free dim (one PSUM bank)
NSUB = CHUNK // SUB   # 2
XCOLS = CHUNK + 2 * PAD  # chunk + halo
TP = T + 2 * PAD      # padded x length in DRAM (zero halos baked in)
NBLK = 10             # contraction blocks: 2 ci-blocks x 5 taps


NWARM_DEFAULT = 60


def _build_nc(reps=1, nwarm=None):
    f32 = mybir.dt.float32
    f16 = mybir.dt.float16
    f8 = mybir.dt.float8e4
    DR = mybir.MatmulPerfMode.DoubleRow

    nc = bass.Bass(trn_type="TRN2", debug=False)
    # xq[p, b, slot, t] : X{A,B}[b*128+p, t-2], zero-padded halos
    xq = nc.dram_tensor("xq", [128, 2, 2, TP], f8, kind="ExternalInput").ap()
    # wts[ci, ((co*10 + k*2+b)*2+slot)*128 + m]  (co-major: lets the first
    # co-block's weights arrive before the full tensor)
    wts = nc.dram_tensor("wts", [128, NBLK * 2 * 2 * 128], f8,
                         kind="ExternalInput").ap()
    bias = nc.dram_tensor("bias", [128, 2], f32, kind="ExternalInput").ap()
    # one output tensor per 2048-wide super-chunk, stored via SWDGE so each
    # store owns a DMASW lane exactly once; host concatenates
    # fp16 output halves store traffic; host upcasts (adds ~3e-4 rel err).
    # The last super-chunk is split into 3 tensors so each tail store owns
    # its own DMA lane (single sync wait each).
    ys = [nc.dram_tensor(f"y{s}", [CI, 2 * CHUNK], f16, kind="ExternalOutput").ap()
          for s in range(NCHUNK // 2 - 1)]
    ys.append(nc.dram_tensor("y7a", [CI, CHUNK], f16, kind="ExternalOutput").ap())
    ys.append(nc.dram_tensor("y7b", [CI, SUB], f16, kind="ExternalOutput").ap())
    ys.append(nc.dram_tensor("y7c", [CI, SUB], f16, kind="ExternalOutput").ap())
    ysab = [yc.rearrange("(b p) t -> p b t", p=128) for yc in ys]

    NOB = 8   # out staging buffers: one per super-chunk, never recycled
    NPB = 6   # psum accumulation banks
    WHALF = NBLK * 2 * 128  # weight columns per co-block
    NWARM = NWARM_DEFAULT if nwarm is None else nwarm
    C0SPLIT = 516           # first piece of chunk-0's load: covers ts=0
                            # exactly, and both pieces stay >=512B/descriptor
                            # (sub-512B runs pay 2x DMA latency)

    with tile.TileContext(nc) as tc:
        with tc.tile_pool(name="wp", bufs=1) as wp, \
             tc.tile_pool(name="pp", bufs=1, space="PSUM") as pp:

            # write-once observer scratch: two columns per observer matmul;
            # warm_ps shares the same PSUM bank
            obs_ps = pp.tile([128, 64], f32, name="obs_ps", tag="obs")
            warm_ps = pp.tile([128, 64], f32, name="warm_ps", tag="warm")
            pbufs = [pp.tile([128, SUB], f32, name=f"pt{j}", tag=f"pt{j}")
                     for j in range(NPB)]
            # x is fully resident: one dedicated buffer per chunk, no reuse
            xbufs = [wp.tile([128, 2, 2, XCOLS], f8, name=f"xt{j}", tag=f"xt{j}")
                     for j in range(NCHUNK)]
            obufs = [wp.tile([128, 2, 2 * CHUNK], f16, name=f"ot{j}", tag=f"ot{j}")
                     for j in range(NOB)]
            # write-once DVE gate scratch: one column per gate memset
            gs = wp.tile([128, 4 * NCHUNK * reps + 8], f32, name="gs")
            # pool-ring gate scratch (absorbs SWDGE lane-reuse waits)
            gsp = wp.tile([128, 4], f32, name="gsp")

            wr = wp.tile([128, NBLK * 2 * 2 * 128], f8, name="wr")
            bs = wp.tile([128, 2], f32, name="bs")
            # warmup scratch: zeroed by DVE, then NWARM tiny DoubleRow
            # matmuls keep PE continuously busy through the p-state ramp
            # while the first loads land, so real matmuls start at 2.4 GHz
            dum = wp.tile([128, 2, 192], f8, name="dum")
            ms_dum = nc.vector.memset(dum[:], 0.0)
            warm = None
            for _ in range(NWARM):
                warm = nc.tensor.matmul(
                    warm_ps[:], dum[:, :, 0:128], dum[:, :, 128:192],
                    start=True, stop=True,
                    perf_mode=mybir.MatmulPerfMode.DoubleRow)


            n_obs = [0]

            def pe_observe(src_ap, dma_inst):
                """2-column matmul whose only wait is `dma_inst`'s lane.

                Reads only within the region `dma_inst` wrote; writes its own
                never-reused obs_ps column (no WAW chain)."""
                n = src_ap.shape[-1]
                m = min(128, n)
                oc = 2 * n_obs[0]
                n_obs[0] += 1
                mm = nc.tensor.matmul(obs_ps[0:m, oc:oc + 2], src_ap[:, 0:m],
                                      src_ap[:, 0:2], start=True, stop=True)
                add_dep_helper(mm.ins, dma_inst.ins, sync=False, reason="obs-order")
                return mm

            n_gate = [0]

            def dve_gate(dep_inst):
                """Write-once DVE memset whose only wait is dep's proc tick."""
                gc = n_gate[0]
                n_gate[0] += 1
                ms = nc.vector.memset(gs[:, gc:gc + 1], 0.0)
                add_dep_helper(ms.ins, dep_inst.ins, sync=True, reason="dve-gate")
                return ms

            bscratch = wp.tile([128, 2], f32, name="bscratch")
            obs_b = None  # emitted after d_b below

            in_dmas = []      # x-chunk loads
            hw_ring = []      # every HWDGE DMA in issue order (lane = idx%8)
            out_dmas = []     # per super-chunk (final rep only)
            gates = []        # ring-gate instructions needing tail coverage
            sc_evicts = {}    # global super-chunk -> last evict
            sc_ods = {}       # global super-chunk -> out dma
            last_mm = None
            last_evict = None
            pi = 0            # psum bank rotation
            NSC = NCHUNK // 2

            obs_w1 = None
            for r in range(reps):
                for c in range(NCHUNK):
                    xt = xbufs[c]

                    observers = []
                    d_c0b = None
                    if r == 0:
                        if c == 0:
                            # ring order: w(co0), c0-first-piece, c0-rest,
                            # w(co1), bias — each lands just before PE (at
                            # full post-warmup speed) reaches the groups
                            # that need it
                            d_w0 = nc.sync.dma_start(out=wr[:, 0:WHALF],
                                                     in_=wts[:, 0:WHALF])
                            hw_ring.append(d_w0)
                            observers.append(
                                pe_observe(wr[:, 0:WHALF], d_w0))
                            da = nc.sync.dma_start(
                                out=xt[:, :, :, 0:C0SPLIT],
                                in_=xq[:, :, :, 0:C0SPLIT])
                            hw_ring.append(da)
                            in_dmas.append(da)
                            observers.append(
                                pe_observe(xt[:, 0, 0, 0:C0SPLIT], da))
                            d_c0b = nc.sync.dma_start(
                                out=xt[:, :, :, C0SPLIT:XCOLS],
                                in_=xq[:, :, :, C0SPLIT:XCOLS])
                            hw_ring.append(d_c0b)
                            in_dmas.append(d_c0b)
                            d_w1 = nc.sync.dma_start(out=wr[:, WHALF:],
                                                     in_=wts[:, WHALF:])
                            hw_ring.append(d_w1)
                            d_b = nc.sync.dma_start(out=bs[:], in_=bias[:])
                            hw_ring.append(d_b)
                            obs_b = nc.vector.tensor_copy(bscratch[:], bs[:])
                        else:
                            # halo baked into DRAM: one uniform DMA per chunk
                            d = nc.sync.dma_start(
                                out=xt[:],
                                in_=xq[:, :, :, c * CHUNK:c * CHUNK + XCOLS])
                            hw_ring.append(d)
                            in_dmas.append(d)
                            observers.append(pe_observe(xt[:, 0, 0, :], d))

                    sc, half = divmod(c, 2)
                    gsc = r * NSC + sc
                    ot = obufs[gsc % NOB]
                    evict_gates = [obs_b]
                    if half == 0 and gsc >= NOB:
                        # pre-lift the recycled out buffer's history onto
                        # DVE's observed clock: one 1-wait gate per proc
                        old = gsc - NOB
                        evict_gates.append(dve_gate(sc_evicts[old]))
                        if old in sc_ods:
                            evict_gates.append(dve_gate(sc_ods[old]))

                    first_evict_of_chunk = True
                    last_chunk = (r == reps - 1 and c == NCHUNK - 1)
                    if last_chunk:
                        # ts-outer so each 512-col piece finishes across both
                        # co blocks early -> finer-grained final stores
                        group_order = [(0, 0), (1, 0), (0, 1), (1, 1)]
                    else:
                        group_order = [(co, ts) for co in range(2)
                                       for ts in range(NSUB)]
                    for gi, (co, ts) in enumerate(group_order):
                        if r == 0 and c == 0:
                            if co == 1 and obs_w1 is None:
                                # co-1 weights observed between chunk-0's co
                                # halves; d_w1 lands well before PE gets here
                                obs_w1 = pe_observe(wr[:, WHALF:], d_w1)
                                observers = observers + [obs_w1]
                            if ts == 1 and d_c0b is not None:
                                observers = observers + [pe_observe(
                                    xt[:, 0, 0, C0SPLIT:XCOLS], d_c0b)]
                                d_c0b = None
                        if True:
                            pt = pbufs[pi % NPB]
                            pi += 1
                            first = True
                            for b in range(2):
                                for k in range(K):
                                    widx = ((co * NBLK + k * 2 + b) * 2) * 128
                                    mm = nc.tensor.matmul(
                                        pt[:],
                                        wr[:, widx:widx + 256].rearrange(
                                            "p (two m) -> p two m", two=2),
                                        xt[:, b, :,
                                           ts * SUB + k: ts * SUB + k + SUB],
                                        start=first,
                                        stop=(b == 1 and k == K - 1),
                                        perf_mode=DR,
                                    )
                                    if first:
                                        for ob in observers:
                                            add_dep_helper(
                                                mm.ins, ob.ins, sync=False,
                                                reason="order-after-observe")
                                    first = False
                                    last_mm = mm
                            off = half * CHUNK + ts * SUB
                            ev = nc.vector.tensor_scalar(
                                out=ot[:, co, off:off + SUB],
                                in0=pt[:],
                                scalar1=DESCALE,
                                scalar2=bs[:, co:co + 1],
                                op0=mybir.AluOpType.mult,
                                op1=mybir.AluOpType.add,
                            )
                            if first_evict_of_chunk:
                                for g in evict_gates:
                                    add_dep_helper(ev.ins, g.ins, sync=False,
                                                   reason="order-after-gate")
                                first_evict_of_chunk = False
                            last_evict = ev

                    final_sc = (r == reps - 1 and sc == NSC - 1)
                    if final_sc and half == 0:
                        # last super-chunk: store the first chunk's columns as
                        # soon as they're evicted so only 512-col pieces tail
                        od = nc.gpsimd.dma_start(out=ysab[NSC - 1][:],
                                                 in_=ot[:, :, 0:CHUNK])
                        out_dmas.append(od)
                    if half == 1:
                        sc_evicts[gsc] = last_evict
                        if r == reps - 1 and not final_sc:
                            # SWDGE store: own output tensor + own DMASW lane
                            od = nc.gpsimd.dma_start(out=ysab[sc][:], in_=ot[:])
                            sc_ods[gsc] = od
                            out_dmas.append(od)
                        elif final_sc:
                            # 512-col tail pieces. These are SWDGE stores 9
                            # and 10 — their lanes wrap onto stores 0 and 1
                            # (8 DMASW lanes), so a pool-ring gate absorbs
                            # each lane-reuse wait (walrus 1-wait limit).
                            pg1 = nc.gpsimd.memset(gsp[:, 0:1], 0.0)
                            add_dep_helper(pg1.ins, out_dmas[0].ins,
                                           sync=True, reason="lane-gate")
                            gates.append(pg1)
                            od1 = nc.gpsimd.dma_start(
                                out=ysab[NSC][:],
                                in_=ot[:, :, CHUNK:CHUNK + SUB])
                            out_dmas.append(od1)
                            # the very last piece rides the idle ACT HWDGE
                            # ring (lower launch overhead than SWDGE); its
                            # lane wraps onto HWDGE DMA #-8, absorbed by an
                            # ACT-ring gate
                            if len(hw_ring) >= 8:
                                pg2 = nc.scalar.memzero(gsp[:, 1:2])
                                add_dep_helper(pg2.ins,
                                               hw_ring[len(hw_ring) - 8].ins,
                                               sync=True, reason="lane-gate")
                                gates.append(pg2)
                            od2 = nc.scalar.dma_start(
                                out=ysab[NSC + 1][:],
                                in_=ot[:, :, CHUNK + SUB:2 * CHUNK])
                            hw_ring.append(od2)
                            out_dmas.append(od2)
                            sc_ods[gsc] = od2

            # Tail flush: cover every proc with 1-dep sync nops so the final
            # drain carries at most one wait. Ordered by expected completion
            # (the nops run in-order on the sync proc).
            tail_deps = (in_dmas[-8:] + out_dmas[:-2] + gates
                         + [last_mm, last_evict] + out_dmas[-2:])
            for dep in tail_deps:
                nop = nc.sync.nop()
                add_dep_helper(nop.ins, dep.ins, sync=True, reason="tailflush")

    return nc


def check_waits(nc):
    """Return instructions carrying more than one sync wait (walrus limit)."""
    bad = []
    for f in nc.m.functions:
        for bb in f.blocks:
            for inst in bb.instructions:
                si = inst.sync_info
                nw = len(si.on_wait) if si and si.on_wait else 0
                if nw > 1:
                    bad.append((inst.name, type(inst).__name__, nw,
                                [w.ant_name for w in si.on_wait]))
    return bad


def _nudge(q, steps):
    """e4m3 array q moved by `steps` representable values (value order)."""
    bits = q.view(np.uint8)
    mag = (bits & 0x7F).astype(np.int32)
    sign = (bits & 0x80) != 0
    m2 = np.where(sign, mag - steps, mag + steps)
    flip = m2 < 0
    m2 = np.where(flip, -m2, m2)
    s2 = np.where(flip, ~sign, sign)
    m2 = np.clip(m2, 0, 0x7E)
    return ((np.where(s2, 0x80, 0) | m2).astype(np.uint8)).view(E4M3)


def _pair_min(v, b_scale, b_weight, na, nb):
    """Choose e4m3 (A, B) minimizing |A + b_weight*B - (1+b_weight*b_scale)*v|.

    A ~ v, B ~ v*b_scale. Returns (A, B) as e4m3 arrays."""
    v = np.asarray(v, np.float32)
    tgt = (1.0 + b_weight * b_scale) * v
    qa = v.astype(E4M3)
    qb = (v * b_scale).astype(E4M3)
    best = None
    bestA = bestB = None
    for ia in range(-na, na + 1):
        A = _nudge(qa, ia)
        Af = A.astype(np.float32)
        for ib in range(-nb, nb + 1):
            Bv = _nudge(qb, ib)
            err = np.abs(Af + np.float32(b_weight) * Bv.astype(np.float32) - tgt)
            if best is None:
                best, bestA, bestB = err, A.copy(), Bv.copy()
            else:
                m = err < best
                np.copyto(best, err, where=m)
                np.copyto(bestA, A, where=m)
                np.copyto(bestB, Bv, where=m)
    return bestA, bestB


def _pack_weights(conv_w, conv_b, lora_A, lora_B):
    w_eff = conv_w.astype(np.float64) + SCALING * np.einsum(
        "or,rik->oik", lora_B.astype(np.float64),
        lora_A.astype(np.float64).reshape(RANK, CI, K))
    w_eff = w_eff.astype(np.float32)
    # both W slots at full scale; slot B enters the sum with weight 1/8
    # (via x slot B's 1/8 scale), so A + B/8 must track (1+1/8) * w * WS
    wa, wb = _pair_min(w_eff * WS, b_scale=1.0, b_weight=DFRAC, na=1, nb=3)
    # wts[ci, ((co*10 + k*2+b)*2+slot)*128 + m] = W{slot}[co*128+m, b*128+ci, k]
    a = np.stack([np.asarray(wa), np.asarray(wb)])     # [slot, CO, CI, K]
    a = a.reshape(2, 2, 128, 2, 128, K)                # [slot, cob, m, cib, ci, k]
    a = a.transpose(4, 1, 5, 3, 0, 2)                  # [ci, cob, k, cib, slot, m]
    wts = np.ascontiguousarray(a.reshape(128, NBLK * 2 * 2 * 128))
    bias = np.ascontiguousarray(
        conv_b.astype(np.float32).reshape(2, 128).T)   # [128, 2]
    return wts, bias


def _pack_x(x):
    """x [CI, T] fp32 -> xq [128, 2, 2, TP] e4m3 with zero halos."""
    xa, xb = _pair_min(x * XS, b_scale=DFRAC, b_weight=1.0, na=1, nb=2)
    xq = np.zeros((128, 2, 2, TP), dtype=E4M3)
    xq[:, :, 0, PAD:PAD + T] = np.asarray(xa).reshape(2, 128, T).transpose(1, 0, 2)
    xq[:, :, 1, PAD:PAD + T] = np.asarray(xb).reshape(2, 128, T).transpose(1, 0, 2)
    return xq


_CACHED_NC = None


def kernel(x, conv_w, conv_b, lora_A, lora_B, _trace=False):
    global _CACHED_NC
    x = np.asarray(x, dtype=np.float32)
    wts, bias = _pack_weights(np.asarray(conv_w), np.asarray(conv_b),
                              np.asarray(lora_A), np.asarray(lora_B))

    if _CACHED_NC is None:
        _CACHED_NC = _build_nc()
        bad = check_waits(_CACHED_NC)
        assert not bad, f"sync-wait violations: {bad[:5]}"
    nc = _CACHED_NC

    in_maps = [
        {"xq": _pack_x(x[i]), "wts": wts, "bias": bias}
        for i in range(N_CORES)
    ]
    res = bass_utils.run_bass_kernel_spmd(
        nc, in_maps, core_ids=list(range(N_CORES)), trace=_trace)
    names = [f"y{s}" for s in range(NCHUNK // 2 - 1)] + ["y7a", "y7b", "y7c"]
    out = np.stack(
        [np.concatenate(
            [res.results[i][nm].astype(np.float32) for nm in names], axis=1)
         for i in range(N_CORES)], axis=0)
    if _trace:
        kernel._last_exec_time_ns = res.exec_time_ns
        kernel._last_results = res
    return out


if __name__ == "__main__":
    nc = _build_nc()
    bad = check_waits(nc)
    print("violations:", bad[:10])
    n_inst = sum(len(bb.instructions) for f in nc.m.functions for bb in f.blocks)
    print("instructions:", n_inst)
